# revision 1
# baseline (speedup 1.0000x reference)
"""Trainium2 Bass kernel for an AttNHP transformer layer.

Shapes (hardcoded): src (4, 1024, 512), nhead=8 with full-width (512) q/k per
head, dim_value 64, ffn 2048.  Runs SPMD on 8 NeuronCores: core c handles
batch c//2, query-token half c%2.  The host rotates the token axis per core so
one shared program works for every core, computes the combined additive
attention-mask bias, and pre-transposes all weights; the device kernel works
entirely in a feature-on-partition (transposed) layout so no on-chip
transposes are needed.
"""

import os
import sys
import numpy as np
from contextlib import ExitStack

for _p in ("/opt/trn_rl_repo", "/root/.axon_site/_ro/trn_rl_repo"):
    if os.path.isdir(_p) and _p not in sys.path:
        sys.path.append(_p)

import concourse.bass as bass  # noqa: E402
import concourse.tile as tile  # noqa: E402
from concourse import bacc, mybir  # noqa: E402
from concourse.bass_utils import run_bass_kernel_spmd  # noqa: E402

f32 = mybir.dt.float32
f32r = mybir.dt.float32r
bf16 = mybir.dt.bfloat16
AF = mybir.ActivationFunctionType
ALU = mybir.AluOpType

B, L, D, H, DV, F = 4, 1024, 512, 8, 64, 2048
LQ = L // 2          # tokens per core
NCORES = 8
NDT = D // 128       # 4  d-tiles
NMT = L // 128       # 8  key-token tiles
NFT = F // 128       # 16 ffn tiles
SCALE = 1.0 / float(np.sqrt(np.float32(D)))
NINF = -1000000.0

_PROG_CACHE = {}
_ACT_PATCHED = [False]


def _patch_act_tables():
    """Force every ACTIVATE onto the natural_log_exp_and_others table set.

    The default chooser maps each function to its first-containing set
    (Exp -> exp_and_others, Ln -> natural_log), which reloads tables twice
    per attention head.  Emptying every other set (order, hence set ids,
    preserved) makes the one set that contains Exp+Ln+Relu+Identity+Square
    the only candidate: one ACT_TABLE_LOAD for the whole kernel.
    """
    if _ACT_PATCHED[0]:
        return
    orig = bacc.get_activation_tables

    def patched(arch):
        t = dict(orig(arch))
        keep = t.get("natural_log_exp_and_others")
        if not keep:
            return t
        for k in t:
            if k != "natural_log_exp_and_others":
                t[k] = t[k] - keep
        return t

    bacc.get_activation_tables = patched
    _ACT_PATCHED[0] = True


def _build_program():
    key = "prog"
    if key in _PROG_CACHE:
        return _PROG_CACHE[key]
    _patch_act_tables()

    nc = bacc.Bacc("TRN2", target_bir_lowering=False, debug=False,
                   num_devices=NCORES)

    def din(name, shape):
        return nc.dram_tensor(name, list(shape), f32, kind="ExternalInput").ap()

    srcT_d = din("srcT", (D, L))
    biasT_d = din("biasT", (L, LQ))
    wqT_d = din("wqT", (D, H * D))
    wkT_d = din("wkT", (D, H * D))
    wvT_d = din("wvT", (D, H * DV))
    bq_d = din("bq", (128, 32))
    bk_d = din("bk", (128, 32))
    bvbc_d = din("bvbc", (128, H * DV))
    ff1T_d = din("ff1T", (D, F))
    ff2T_d = din("ff2T", (F, D))
    ff1b_d = din("ff1b", (128, NFT))
    ff2b_d = din("ff2b", (128, NDT))
    ln2g_d = din("ln2g", (128, NDT))
    ln2b_d = din("ln2b", (128, NDT))
    outT_d = nc.dram_tensor("outT", [D, LQ], f32, kind="ExternalOutput").ap()

    with tile.TileContext(nc) as tc, ExitStack() as ctx:
        pp = ctx.enter_context(tc.tile_pool(name="pp", bufs=1))
        ffw1 = ctx.enter_context(tc.tile_pool(name="ffw1", bufs=8))
        ffw2 = ctx.enter_context(tc.tile_pool(name="ffw2", bufs=6))
        ps = ctx.enter_context(tc.tile_pool(name="ps", bufs=8, space="PSUM"))
        ap_x1 = ctx.enter_context(tc.tile_pool(name="x1p", bufs=1))
        ap_sq = ctx.enter_context(tc.tile_pool(name="sqp", bufs=2))

        def load_const(name, dram, shape):
            t = pp.tile(list(shape), f32, name=name, tag=name)
            nc.sync.dma_start(t[:], dram[:])
            return t

        # small consts needed early
        bq_sb = load_const("bq", bq_d, (128, 32))
        bk_sb = load_const("bk", bk_d, (128, 32))
        onesf = pp.tile([128, 8], f32, name="onesf", tag="onesf")
        nc.vector.memset(onesf[:], 1.0)
        ones_col = pp.tile([128, 1], f32r, name="ones", tag="ones")
        nc.vector.tensor_copy(ones_col[:], onesf[:, 0:1])
        onesr = pp.tile([1, 128], f32, name="onesr", tag="onesr")
        nc.vector.memset(onesr[:], 1.0)
        ones_row = pp.tile([1, 128], f32r, name="onesrr", tag="onesrr")
        nc.vector.tensor_copy(ones_row[:], onesr[:])
        eps30 = pp.tile([1, 1], f32, name="eps30", tag="eps30")
        nc.vector.memset(eps30[:], 1e-30)
        epsln = pp.tile([1, 1], f32, name="epsln", tag="epsln")
        nc.vector.memset(epsln[:], 1e-5)

        srcT = []
        for dt in range(NDT):
            t = pp.tile([128, L], f32r, name=f"srcT{dt}", tag=f"srcT{dt}")
            nc.sync.dma_start(t[:, 0:LQ],
                              srcT_d[dt * 128:(dt + 1) * 128, 0:LQ].bitcast(f32r))
            srcT.append(t)


        vaug = [pp.tile([128, H * 65], f32r, name=f"vaug{mt}", tag=f"vaug{mt}")
                for mt in range(NMT)]
        saT = [pp.tile([128, LQ], f32, name=f"saT{dt}", tag=f"saT{dt}")
               for dt in range(NDT)]
        biasT = [pp.tile([128, LQ], f32, name=f"biasT{mt}", tag=f"biasT{mt}")
                 for mt in range(NMT)]

        with ExitStack() as actx:
            ap_wq = actx.enter_context(tc.tile_pool(name="wqp", bufs=6))
            ap_wk = actx.enter_context(tc.tile_pool(name="wkp", bufs=6))
            ap_qh = actx.enter_context(tc.tile_pool(name="qhp", bufs=8))
            ap_kh = actx.enter_context(tc.tile_pool(name="khp", bufs=8))
            ap_ex = actx.enter_context(tc.tile_pool(name="exq", bufs=8))
            ap_sc = actx.enter_context(tc.tile_pool(name="scq", bufs=4))
            ap_row = actx.enter_context(tc.tile_pool(name="rowp", bufs=2))

            def emit_proj(h, mid_emit=None):
                wq_t, wk_t = [], []
                for dt in range(NDT):
                    t = ap_wq.tile([128, D], f32r, name="wq", tag="wq")
                    nc.sync.dma_start(
                        t[:], wqT_d[dt * 128:(dt + 1) * 128,
                                    h * D:(h + 1) * D].bitcast(f32r))
                    wq_t.append(t)
                for dt in range(NDT):
                    t = ap_wk.tile([128, D], f32r, name="wk", tag="wk")
                    nc.sync.dma_start(
                        t[:], wkT_d[dt * 128:(dt + 1) * 128,
                                    h * D:(h + 1) * D].bitcast(f32r))
                    wk_t.append(t)
                qh = []
                for pt in range(NDT):
                    pq = ps.tile([128, LQ], f32, name="psw", tag="ps")
                    for dt in range(NDT):
                        nc.tensor.matmul(pq[:], wq_t[dt][:, pt * 128:(pt + 1) * 128],
                                         srcT[dt][:, 0:LQ],
                                         start=(dt == 0), stop=(dt == NDT - 1))
                    qt = ap_qh.tile([128, LQ], bf16, name="qh", tag="qh")
                    nc.vector.tensor_scalar(qt[:], pq[:],
                                            bq_sb[:, h * 4 + pt:h * 4 + pt + 1],
                                            None, ALU.add)
                    qh.append(qt)
                if mid_emit is not None:
                    mid_emit()
                kh = [ap_kh.tile([128, L], bf16, name="kh", tag="kh")
                      for _ in range(NDT)]
                for nh in range(2):
                    for pt in range(NDT):
                        pk = ps.tile([128, LQ], f32, name="psw", tag="ps")
                        for dt in range(NDT):
                            nc.tensor.matmul(
                                pk[:], wk_t[dt][:, pt * 128:(pt + 1) * 128],
                                srcT[dt][:, nh * LQ:(nh + 1) * LQ],
                                start=(dt == 0), stop=(dt == NDT - 1))
                        nc.vector.tensor_scalar(
                            kh[pt][:, nh * LQ:(nh + 1) * LQ], pk[:],
                            bk_sb[:, h * 4 + pt:h * 4 + pt + 1], None, ALU.add)
                return qh, kh

            def emit_scores(h, qh, kh):
                ex = []
                for mt in range(NMT):
                    psc = ps.tile([128, LQ], f32, name="psw", tag="ps")
                    for pt in range(NDT):
                        nc.tensor.matmul(psc[:], kh[pt][:, mt * 128:(mt + 1) * 128],
                                         qh[pt][:],
                                         start=(pt == 0), stop=(pt == NDT - 1))
                    sct = ap_sc.tile([128, LQ], f32, name="sc", tag="sc")
                    nc.vector.tensor_tensor(sct[:], psc[:], biasT[mt][:], ALU.add)
                    et = ap_ex.tile([128, LQ], f32r, name="ex", tag="ex")
                    nc.scalar.activation(et[:], sct[:], AF.Exp, scale=SCALE)
                    ex.append(et)
                return ex

            def emit_pv_mm(h, ex):
                ppv = ps.tile([65, LQ], f32, name="ppv", tag="ps")
                for mt in range(NMT):
                    nc.tensor.matmul(ppv[:], vaug[mt][:, h * 65:(h + 1) * 65],
                                     ex[mt][:], start=(mt == 0), stop=(mt == NMT - 1))
                lt = ap_row.tile([1, LQ], f32, name="lt", tag="lt")
                nc.scalar.activation(lt[:], ppv[64:65, :], AF.Ln, bias=eps30[:])
                rt = ap_row.tile([1, LQ], f32r, name="rt", tag="rt")
                nc.scalar.activation(rt[:], lt[:], AF.Exp, scale=-1.0)
                return ppv, rt

            def emit_norm(h, ppv, rt):
                prb = ps.tile([64, LQ], f32, name="prb", tag="ps")
                nc.tensor.matmul(prb[:], ones_row[0:1, 0:64], rt[:],
                                 start=True, stop=True)
                rbc = ap_row.tile([64, LQ], f32, name="rbc", tag="rbc")
                nc.vector.tensor_copy(rbc[:], prb[:])
                sat = saT[h // 2]
                r0 = (h % 2) * 64
                nc.vector.tensor_tensor(sat[r0:r0 + 64, :], ppv[0:64, :], rbc[:],
                                        ALU.mult)

            # ---- startup-ordered emission ----
            def _late_src():
                for dt in range(NDT):
                    nc.sync.dma_start(
                        srcT[dt][:, LQ:L],
                        srcT_d[dt * 128:(dt + 1) * 128, LQ:L].bitcast(f32r))

            qh0, kh0 = emit_proj(0, mid_emit=_late_src)

            # V projection (natural layout [m, j]) + ones column
            wv = []
            for dt in range(NDT):
                t = pp.tile([128, H * DV], f32r, name=f"wv{dt}", tag=f"wv{dt}")
                nc.sync.dma_start(t[:],
                                  wvT_d[dt * 128:(dt + 1) * 128, :].bitcast(f32r))
                wv.append(t)
            bvbc_sb = load_const("bvbc", bvbc_d, (128, H * DV))
            for mt in range(NMT):
                pv = ps.tile([128, H * DV], f32, name="psv", tag="ps")
                for dt in range(NDT):
                    nc.tensor.matmul(pv[:], srcT[dt][:, mt * 128:(mt + 1) * 128],
                                     wv[dt][:], start=(dt == 0), stop=(dt == NDT - 1))
                va_v = vaug[mt][:].rearrange("p (h c) -> p h c", c=65)[:, :, 0:64]
                pv_v = pv[:].rearrange("p (h c) -> p h c", c=64)
                bv_v = bvbc_sb[:].rearrange("p (h c) -> p h c", c=64)
                nc.vector.tensor_tensor(va_v, pv_v, bv_v, ALU.add)
                va_ones = vaug[mt][:].rearrange("p (h c) -> p h c", c=65)[:, :, 64:65]
                nc.vector.tensor_copy(va_ones,
                                      onesf[:].rearrange("p (h o) -> p h o", o=1))

            # remaining big/late loads
            for mt in range(NMT):
                nc.sync.dma_start(biasT[mt][:],
                                  biasT_d[mt * 128:(mt + 1) * 128, :])
            ff1b_sb = load_const("ff1b", ff1b_d, (128, NFT))
            ff2b_sb = load_const("ff2b", ff2b_d, (128, NDT))
            ln2g_sb = load_const("ln2g", ln2g_d, (128, NDT))
            ln2b_sb = load_const("ln2b", ln2b_d, (128, NDT))

            # heads pipelined: scores(h) -> norm(h-1) -> proj(h+1) -> pv(h)
            x1 = [None] * NDT
            sqs = [None] * NDT
            qk = (qh0, kh0)
            pend = None

            def flush_norm():
                hp_, ppv_, rt_ = pend
                emit_norm(hp_, ppv_, rt_)
                if hp_ % 2 == 1:
                    dt = hp_ // 2
                    t = ap_x1.tile([128, LQ], f32r, name=f"x1{dt}", tag=f"x1{dt}")
                    nc.vector.tensor_tensor(t[:], srcT[dt][:, 0:LQ].bitcast(f32),
                                            saT[dt][:], ALU.add)
                    x1[dt] = t
                    sqt = ap_sq.tile([128, LQ], f32r, name=f"sq{dt}",
                                     tag=f"sq{dt}", bufs=1)
                    nc.scalar.activation(sqt[:], t[:].bitcast(f32), AF.Square)
                    sqs[dt] = sqt

            for h in range(H):
                ex = emit_scores(h, *qk)
                if pend is not None:
                    flush_norm()
                qk = emit_proj(h + 1) if h + 1 < H else None
                ppv, rt = emit_pv_mm(h, ex)
                pend = (h, ppv, rt)
            flush_norm()

            # LN1 stats matmuls (kept out of the mid-attention PE stream)
            psx = ps.tile([1, LQ], f32, name="pstx", tag="ps")
            pss = ps.tile([1, LQ], f32, name="psts", tag="ps")
            for dt in range(NDT):
                nc.tensor.matmul(psx[:], ones_col[:], x1[dt][:],
                                 start=(dt == 0), stop=(dt == NDT - 1))
                nc.tensor.matmul(pss[:], ones_col[:], sqs[dt][:],
                                 start=(dt == 0), stop=(dt == NDT - 1))

        # LN1 tail (stats -> mean/rstd -> apply)
        if True:
            with ExitStack() as fctx:
                fpp = fctx.enter_context(tc.tile_pool(name="fpp", bufs=1))
                fp = fctx.enter_context(tc.tile_pool(name="fp", bufs=2))
                hp = fctx.enter_context(tc.tile_pool(name="hp", bufs=4))

                def ln_tail(psx_, pss_, x_tiles, g_sb, b_sb, out_dtype,
                            out_tag_prefix, out_pool):
                    comb = fp.tile([1, 2 * LQ], f32r, name="comb", tag="comb")
                    mean = comb[0:1, 0:LQ]
                    nc.vector.tensor_scalar(mean, psx_[:], 1.0 / D, None,
                                            ALU.mult)
                    m2 = fp.tile([1, LQ], f32, name="m2", tag="m2")
                    nc.vector.tensor_tensor(m2[:], mean.bitcast(f32),
                                            mean.bitcast(f32), ALU.mult)
                    var = fp.tile([1, LQ], f32, name="var", tag="var")
                    nc.vector.scalar_tensor_tensor(var[:], pss_[:], 1.0 / D,
                                                   m2[:], ALU.mult, ALU.subtract)
                    lnv = fp.tile([1, LQ], f32, name="lnv", tag="lnv")
                    nc.scalar.activation(lnv[:], var[:], AF.Ln, bias=epsln[:])
                    nc.scalar.activation(comb[0:1, LQ:2 * LQ], lnv[:], AF.Exp,
                                         scale=-0.5)
                    pmb = ps.tile([128, LQ], f32, name="pmb", tag="ps")
                    nc.tensor.matmul(pmb[:], ones_row[:], comb[0:1, 0:LQ],
                                     start=True, stop=True)
                    prs = ps.tile([128, LQ], f32, name="prs", tag="ps")
                    nc.tensor.matmul(prs[:], ones_row[:], comb[0:1, LQ:2 * LQ],
                                     start=True, stop=True)
                    outs = []
                    for dt in range(NDT):
                        t1 = fp.tile([128, LQ], f32, name="lnt1", tag="lnt1")
                        nc.vector.tensor_tensor(t1[:], x_tiles[dt][:].bitcast(f32),
                                                pmb[:], ALU.subtract)
                        if g_sb is None:
                            o = out_pool.tile([128, LQ], out_dtype,
                                              name=f"{out_tag_prefix}{dt}",
                                              tag=f"{out_tag_prefix}{dt}")
                            nc.vector.tensor_tensor(o[:], t1[:], prs[:], ALU.mult)
                        else:
                            t2 = fp.tile([128, LQ], f32, name="lnt2", tag="lnt2")
                            nc.vector.tensor_tensor(t2[:], t1[:], prs[:], ALU.mult)
                            o = out_pool.tile([128, LQ], out_dtype,
                                              name=f"{out_tag_prefix}{dt}",
                                              tag=f"{out_tag_prefix}{dt}")
                            nc.scalar.activation(o[:], t2[:], AF.Identity,
                                                 bias=b_sb[:, dt:dt + 1],
                                                 scale=g_sb[:, dt:dt + 1])
                        outs.append(o)
                    return outs

                xT = ln_tail(psx, pss, x1, None, None, f32r, "xT", fpp)

                # FFN
                pf2 = [ps.tile([128, LQ], f32, name=f"pf2_{i}", tag="ps")
                       for i in range(NDT)]
                f1t = None
                for ft in range(NFT):
                    fs, fi = divmod(ft, 4)
                    if fi == 0:
                        f1t = []
                        for dt in range(NDT):
                            t = ffw1.tile([128, 512], f32r, name="ff1w", tag="ff1w")
                            nc.sync.dma_start(
                                t[:], ff1T_d[dt * 128:(dt + 1) * 128,
                                             fs * 512:(fs + 1) * 512].bitcast(f32r))
                            f1t.append(t)
                    ph1 = ps.tile([128, LQ], f32, name="ph1", tag="ps")
                    for dt in range(NDT):
                        nc.tensor.matmul(ph1[:], f1t[dt][:, fi * 128:(fi + 1) * 128],
                                         xT[dt][:],
                                         start=(dt == 0), stop=(dt == NDT - 1))
                    h1t = hp.tile([128, LQ], f32r, name="h1", tag="h1")
                    nc.scalar.activation(h1t[:], ph1[:], AF.Relu,
                                         bias=ff1b_sb[:, ft:ft + 1])
                    f2t = ffw2.tile([128, D], f32r, name="ff2w", tag="ff2w")
                    nc.sync.dma_start(
                        f2t[:], ff2T_d[ft * 128:(ft + 1) * 128, :].bitcast(f32r))
                    for dot in range(NDT):
                        nc.tensor.matmul(pf2[dot][:],
                                         f2t[:, dot * 128:(dot + 1) * 128],
                                         h1t[:], start=(ft == 0),
                                         stop=(ft == NFT - 1))

                # ff2 bias + residual, LN2 stats
                x2 = []
                psx2 = ps.tile([1, LQ], f32, name="pstx2", tag="ps")
                pss2 = ps.tile([1, LQ], f32, name="psts2", tag="ps")
                for dot in range(NDT):
                    t = fpp.tile([128, LQ], f32r, name=f"x2{dot}", tag=f"x2{dot}")
                    nc.vector.scalar_tensor_tensor(t[:], pf2[dot][:],
                                                   ff2b_sb[:, dot:dot + 1],
                                                   xT[dot][:].bitcast(f32),
                                                   ALU.add, ALU.add)
                    x2.append(t)
                    nc.tensor.matmul(psx2[:], ones_col[:], t[:],
                                     start=(dot == 0), stop=(dot == NDT - 1))
                    sqt = ap_sq.tile([128, LQ], f32r, name="sqB", tag="sqB")
                    nc.scalar.activation(sqt[:], t[:].bitcast(f32), AF.Square)
                    nc.tensor.matmul(pss2[:], ones_col[:], sqt[:],
                                     start=(dot == 0), stop=(dot == NDT - 1))

                outs = ln_tail(psx2, pss2, x2, ln2g_sb, ln2b_sb, f32, "outp", fp)
                for dt in range(NDT):
                    nc.sync.dma_start(outT_d[dt * 128:(dt + 1) * 128, :],
                                      outs[dt][:])

    nc.compile()
    _PROG_CACHE[key] = nc
    return nc


def _col_tiles(vec):
    """(N,) -> (128, N//128) with [p, j] = vec[j*128 + p]."""
    return np.ascontiguousarray(vec.reshape(-1, 128).T.astype(np.float32))


def _prep_inputs(src, mask, attn_mask, in_proj_w, in_proj_b, ln1_g, ln1_b,
                 ff1_w, ff1_b, ff2_w, ff2_b, ln2_g, ln2_b):
    src = np.asarray(src, np.float32)
    mask = np.asarray(mask, bool)
    attn_mask = np.asarray(attn_mask, bool)
    wqT = np.ascontiguousarray(np.asarray(in_proj_w[:H * D], np.float32).T)
    wkT = np.ascontiguousarray(np.asarray(in_proj_w[H * D:2 * H * D], np.float32).T)
    wvT = np.ascontiguousarray(np.asarray(in_proj_w[2 * H * D:], np.float32).T)
    b = np.asarray(in_proj_b, np.float32)
    bq = _col_tiles(b[:H * D])
    bk = _col_tiles(b[H * D:2 * H * D])
    bvbc = np.ascontiguousarray(
        np.broadcast_to(b[2 * H * D:].astype(np.float32), (128, H * DV)))
    shared = {
        "wqT": wqT, "wkT": wkT, "wvT": np.ascontiguousarray(wvT),
        "bq": bq, "bk": bk, "bvbc": bvbc,
        "ff1T": np.ascontiguousarray(
            (np.asarray(ff1_w, np.float64) * np.asarray(ln1_g, np.float64)[None, :]
             ).T.astype(np.float32)),
        "ff2T": np.ascontiguousarray(np.asarray(ff2_w, np.float32).T),
        "ff1b": _col_tiles(
            (np.asarray(ff1_w, np.float64) @ np.asarray(ln1_b, np.float64)
             + np.asarray(ff1_b, np.float64)).astype(np.float32)),
        "ff2b": _col_tiles(np.asarray(ff2_b, np.float32)),
        "ln2g": _col_tiles(np.asarray(ln2_g, np.float32)),
        "ln2b": _col_tiles(np.asarray(ln2_b, np.float32)),
    }
    in_maps = []
    for c in range(NCORES):
        bidx, half = divmod(c, 2)
        perm = np.r_[half * LQ:L, 0:half * LQ]
        srcT = np.ascontiguousarray(src[bidx].T[:, perm])
        # combined additive mask bias, transposed to [m_rot, l_own]
        cm = mask[bidx][None, :] | attn_mask[half * LQ:(half + 1) * LQ, :]
        biasT = np.ascontiguousarray(
            (cm[:, perm].T.astype(np.float32) * NINF))
        m = dict(shared)
        m["srcT"] = srcT
        m["biasT"] = biasT
        in_maps.append(m)
    return in_maps


def _run(inputs, trace=False):
    nc = _build_program()
    in_maps = _prep_inputs(**inputs)
    for attempt in range(3):
        try:
            res = run_bass_kernel_spmd(nc, in_maps, list(range(NCORES)),
                                       trace=trace)
            break
        except Exception:  # transient NRT device errors observed
            if attempt == 2:
                raise
    out = np.empty((B, L, D), np.float32)
    for c in range(NCORES):
        bidx, half = divmod(c, 2)
        out[bidx, half * LQ:(half + 1) * LQ, :] = res.results[c]["outT"].T
    return out, res


def kernel(**inputs):
    out, _ = _run(inputs, trace=False)
    return out


if __name__ == "__main__":
    import reference
    inputs = {k: np.asarray(v) for k, v in reference.setup_inputs().items()}
    out = kernel(**inputs)
    print("out", out.shape, out.dtype)



# revision 26
# speedup vs baseline: 1.7394x; 1.7394x over previous
"""Trainium2 Bass kernel for an AttNHP transformer layer (v2, fp8 DoubleRow).

Shapes (hardcoded): src (4, 1024, 512), nhead=8 with full-width (512) q/k per
head, dim_value 64, ffn 2048.  SPMD on 8 NeuronCores: core c handles batch
c//2, query-token half c%2.

Key algebra (host-folded):
  scores[l,m] = q_l.k_m = s_l^T (Wq^T Wk) s_m + (Wk^T bq).s_m + f(l)
The f(l) terms shift all logits of a query equally and cancel in softmax, so
with M_h = Wq_h^T Wk_h and u_h = Wk_h^T bq_h (both host-precomputed):
  qhat_l = M_h^T-applied projection of own queries (+u_h bias), keys enter as
  raw src.  This removes the Q/K projections entirely.  qhat/scores/V run as
fp8e4 DoubleRow matmuls (2x PE rate, 256-deep contraction per pass); the
pad+attn mask is one multiplicative bf16 {0,1} tensor applied post-exp; FFN
runs in bf16.  Per-head softmax denominators are batched into one Ln/Exp pair.
"""

import os
import sys
import numpy as np
from contextlib import ExitStack

for _p in ("/opt/trn_rl_repo", "/root/.axon_site/_ro/trn_rl_repo"):
    if os.path.isdir(_p) and _p not in sys.path:
        sys.path.append(_p)

import ml_dtypes  # noqa: E402
import concourse.bass as bass  # noqa: E402
import concourse.tile as tile  # noqa: E402
from concourse import bacc, mybir  # noqa: E402
from concourse.bass_utils import run_bass_kernel_spmd  # noqa: E402

f32 = mybir.dt.float32
f32r = mybir.dt.float32r
bf16 = mybir.dt.bfloat16
f8 = mybir.dt.float8e4
AF = mybir.ActivationFunctionType
ALU = mybir.AluOpType
DR = mybir.MatmulPerfMode.DoubleRow
np_f8 = ml_dtypes.float8_e4m3fn
np_bf16 = ml_dtypes.bfloat16

B, L, D, H, DV, F = 4, 1024, 512, 8, 64, 2048
LQ = L // 2          # own query tokens per core
NCORES = 8
NDT = D // 128       # 4
NMT = L // 128       # 8
NFT = F // 128       # 16
SM = 16.0            # host prescale on M (fp8 range)
SCALE = 1.0 / (SM * float(np.sqrt(np.float32(D))))
NINF = -1000000.0

_PROG_CACHE = {}
_ACT_PATCHED = [False]


def _patch_act_tables():
    """Force every ACTIVATE onto natural_log_exp_and_others (one table load)."""
    if _ACT_PATCHED[0]:
        return
    orig = bacc.get_activation_tables

    def patched(arch):
        t = dict(orig(arch))
        keep = t.get("natural_log_exp_and_others")
        if not keep:
            return t
        for k in t:
            if k != "natural_log_exp_and_others":
                t[k] = t[k] - keep
        return t

    bacc.get_activation_tables = patched
    _ACT_PATCHED[0] = True


def _build_program():
    key = "prog"
    if key in _PROG_CACHE:
        return _PROG_CACHE[key]
    _patch_act_tables()

    nc = bacc.Bacc("TRN2", target_bir_lowering=False, debug=False,
                   num_devices=NCORES)

    def din(name, shape, dt):
        return nc.dram_tensor(name, list(shape), dt, kind="ExternalInput").ap()

    src8_d = din("src8", (128, NDT, L), f8)         # [pi, po, m] = src[po*128+pi, m]
    srcbf_d = din("srcbf", (D, LQ), bf16)           # own-half srcT for residual
    m8_d = din("m8", (128, H * NDT, D), f8)         # per head: [pi, po, d] of 16*M_h
    u16_d = din("u16", (128, H * NDT), f32)         # 16*u_h[po*128+pi]
    maskf_d = din("maskf", (128, NMT, LQ), bf16)    # (1-pad)(1-attn) as {0,1}
    wv8_d = din("wv8", (128, NDT, H * DV), f8)      # [pi, po, j]
    bvbc_d = din("bvbc", (128, H * DV), f32)
    ff1T_d = din("ff1T", (D, F), bf16)
    ff2T_d = din("ff2T", (F, D), bf16)
    ff1b_d = din("ff1b", (128, NFT), f32)
    ff2b_d = din("ff2b", (128, NDT), f32)
    ln2g_d = din("ln2g", (128, NDT), f32)
    ln2b_d = din("ln2b", (128, NDT), f32)
    outT_d = nc.dram_tensor("outT", [D, LQ], f32, kind="ExternalOutput").ap()

    with tile.TileContext(nc) as tc, ExitStack() as ctx:
        pp = ctx.enter_context(tc.tile_pool(name="pp", bufs=1))
        hp = ctx.enter_context(tc.tile_pool(name="hp", bufs=2))

        def load_const(name, dram, shape, dt=f32):
            t = pp.tile(list(shape), dt, name=name, tag=name)
            nc.sync.dma_start(t[:], dram[:])
            return t

        # early loads: src8 + M (first heads) start the pipeline
        src8 = pp.tile([128, NDT, L], f8, name="src8", tag="src8")
        nc.sync.dma_start(src8[:], src8_d[:])
        m8 = pp.tile([128, H * NDT, D], f8, name="m8", tag="m8")
        nc.sync.dma_start(m8[:, 0:NDT, :], m8_d[:, 0:NDT, :])
        u16 = load_const("u16", u16_d, (128, H * NDT))

        # small consts
        onesc_bf = pp.tile([128, 1], bf16, name="onescb", tag="onescb")
        nc.vector.memset(onesc_bf[:], 1.0)
        onesr_f = pp.tile([1, 128], f32, name="onesrw", tag="onesrw")
        nc.vector.memset(onesr_f[:], 1.0)
        onesr_fr = pp.tile([1, 128], f32r, name="onesrf", tag="onesrf")
        nc.vector.tensor_copy(onesr_fr[:], onesr_f[:])
        onesr_bf = pp.tile([1, 128], bf16, name="onesrb", tag="onesrb")
        nc.vector.memset(onesr_bf[:], 1.0)
        eps30 = pp.tile([1, 1], f32, name="eps30", tag="eps30")
        nc.vector.memset(eps30[:], 1e-30)
        epsln = pp.tile([1, 1], f32, name="epsln", tag="epsln")
        nc.vector.memset(epsln[:], 1e-5)

        maskf = pp.tile([128, NMT, LQ], bf16, name="maskf", tag="maskf")
        nc.sync.dma_start(maskf[:], maskf_d[:])
        wv8 = pp.tile([128, NDT, H * DV], f8, name="wv8", tag="wv8")
        nc.sync.dma_start(wv8[:], wv8_d[:])
        bvbc = load_const("bvbc", bvbc_d, (128, H * DV))
        srcbf = [pp.tile([128, LQ], bf16, name=f"srcbf{dt}", tag=f"srcbf{dt}")
                 for dt in range(NDT)]
        for dt in range(NDT):
            nc.sync.dma_start(srcbf[dt][:],
                              srcbf_d[dt * 128:(dt + 1) * 128, :])
        # remaining M heads
        nc.sync.dma_start(m8[:, NDT:H * NDT, :], m8_d[:, NDT:H * NDT, :])

        # vaug: [pi, mt, h*65+c]; c==64 is the ones column (softmax denom)
        vaug = pp.tile([128, NMT, H * 65], bf16, name="vaug", tag="vaug")
        v_ones = vaug[:].rearrange("p m (h c) -> p m h c", c=65)[:, :, :, 64:65]
        nc.vector.memset(v_ones, 1.0)

        # persistent activations
        qhat8 = [pp.tile([128, NDT, LQ], f8, name=f"qh8_{i}", tag=f"qh8_{i}")
                 for i in range(2)]
        ex = [pp.tile([128, NMT, LQ], bf16, name=f"ex{i}", tag=f"ex{i}")
              for i in range(2)]
        saU = pp.tile([64, H, LQ], bf16, name="saU", tag="saU")
        dnm = pp.tile([1, H * LQ], bf16, name="dnm", tag="dnm")
        saT = [pp.tile([128, LQ], bf16, name=f"saT{dt}", tag=f"saT{dt}")
               for dt in range(NDT)]
        x1 = [pp.tile([128, LQ], bf16, name=f"x1_{dt}", tag=f"x1_{dt}")
              for dt in range(NDT)]
        sq = [pp.tile([128, LQ], bf16, name=f"sq{dt}", tag=f"sq{dt}")
              for dt in range(NDT)]
        xT = [pp.tile([128, LQ], bf16, name=f"xT{dt}", tag=f"xT{dt}")
              for dt in range(NDT)]
        x2 = [pp.tile([128, LQ], bf16, name=f"x2_{dt}", tag=f"x2_{dt}")
              for dt in range(NDT)]

        # FFN weights (prefetched; consumed after attention)
        ff1w = [pp.tile([128, F], bf16, name=f"f1w{dt}", tag=f"f1w{dt}")
                for dt in range(NDT)]
        ff2w = [pp.tile([128, D], bf16, name=f"f2w{ft}", tag=f"f2w{ft}")
                for ft in range(NFT)]
        for dt in range(NDT):
            nc.sync.dma_start(ff1w[dt][:], ff1T_d[dt * 128:(dt + 1) * 128, :])
        for ft in range(NFT):
            nc.sync.dma_start(ff2w[ft][:], ff2T_d[ft * 128:(ft + 1) * 128, :])
        ff1b = load_const("ff1b", ff1b_d, (128, NFT))
        ff2b = load_const("ff2b", ff2b_d, (128, NDT))
        ln2g = load_const("ln2g", ln2g_d, (128, NDT))
        ln2b = load_const("ln2b", ln2b_d, (128, NDT))

        with ExitStack() as actx:
            ps_qh = actx.enter_context(
                tc.tile_pool(name="psqh", bufs=2, space="PSUM"))
            ps_sc = actx.enter_context(
                tc.tile_pool(name="pssc", bufs=2, space="PSUM"))
            ps_pv = actx.enter_context(
                tc.tile_pool(name="pspv", bufs=2, space="PSUM"))
            fe = actx.enter_context(tc.tile_pool(name="fe", bufs=3))

            def emit_qhat(h):
                """qhat8[h%2][:, po, :] = fp8(psum + 16*u), psum = 16*M_h^T s_own.

                Host rotates the m axis so own query tokens are columns 0:LQ.
                """
                q8 = qhat8[h % 2]
                for po in range(NDT):
                    pq = ps_qh.tile([128, LQ], f32, name="pq", tag="qh")
                    for t in range(NDT // 2):
                        nc.tensor.matmul(
                            pq[:],
                            m8[:, h * NDT + 2 * t:h * NDT + 2 * t + 2,
                               po * 128:(po + 1) * 128],
                            src8[:, 2 * t:2 * t + 2, 0:LQ],
                            start=(t == 0), stop=(t == NDT // 2 - 1),
                            perf_mode=DR)
                    nc.vector.tensor_scalar(
                        q8[:, po, :], pq[:],
                        u16[:, h * NDT + po:h * NDT + po + 1], None, ALU.add)

            def emit_scores_softmax(h):
                """psc pairs -> exp (ACT) -> mask mult (DVE) -> ex[h%2]."""
                exh = ex[h % 2]
                q8 = qhat8[h % 2]
                for p in range(NMT // 2):
                    psc = ps_sc.tile([128, 2 * LQ], f32, name="psc", tag="sc")
                    for half in range(2):
                        mt = 2 * p + half
                        for t in range(NDT // 2):
                            nc.tensor.matmul(
                                psc[:, half * LQ:(half + 1) * LQ],
                                src8[:, 2 * t:2 * t + 2,
                                     mt * 128:(mt + 1) * 128],
                                q8[:, 2 * t:2 * t + 2, :],
                                start=(t == 0), stop=(t == NDT // 2 - 1),
                                perf_mode=DR)
                    exf = fe.tile([128, 2 * LQ], bf16, name="exf", tag="exf")
                    nc.scalar.activation(exf[:], psc[:], AF.Exp, scale=SCALE)
                    nc.vector.tensor_tensor(
                        exh[:, 2 * p:2 * p + 2, :].rearrange("p a b -> p (a b)"),
                        exf[:], maskf[:, 2 * p:2 * p + 2, :].rearrange(
                            "p a b -> p (a b)"), ALU.mult)

            def emit_pv(h):
                """ppv[65, LQ]: rows 0:64 numerator, row 64 denominator."""
                exh = ex[h % 2]
                ppv = ps_pv.tile([65, LQ], f32, name="ppv", tag="pv")
                for mt in range(NMT):
                    nc.tensor.matmul(
                        ppv[:],
                        vaug[:, mt, h * 65:(h + 1) * 65],
                        exh[:, mt, :],
                        start=(mt == 0), stop=(mt == NMT - 1))
                nc.vector.tensor_copy(saU[:, h, :], ppv[0:64, :])
                nc.vector.tensor_copy(dnm[0:1, h * LQ:(h + 1) * LQ],
                                      ppv[64:65, :])

            # V projection (into vaug) -- uses the sc psum pairs before heads
            for p in range(NMT // 2):
                pv = ps_sc.tile([128, 2 * LQ], f32, name="pvv", tag="sc")
                for half in range(2):
                    mt = 2 * p + half
                    for t in range(NDT // 2):
                        nc.tensor.matmul(
                            pv[:, half * LQ:(half + 1) * LQ],
                            src8[:, 2 * t:2 * t + 2, mt * 128:(mt + 1) * 128],
                            wv8[:, 2 * t:2 * t + 2, :],
                            start=(t == 0), stop=(t == NDT // 2 - 1),
                            perf_mode=DR)
                for half in range(2):
                    mt = 2 * p + half
                    va_v = vaug[:, mt, :].rearrange(
                        "p (h c) -> p h c", c=65)[:, :, 0:64]
                    pv_v = pv[:, half * LQ:(half + 1) * LQ].rearrange(
                        "p (h c) -> p h c", c=64)
                    bv_v = bvbc[:].rearrange("p (h c) -> p h c", c=64)
                    nc.vector.tensor_tensor(va_v, pv_v, bv_v, ALU.add)

            # head pipeline: qhat(h+1) first so its DVE quantize lands ahead
            # of mask(h) in the DVE queue (in-order engines).
            emit_qhat(0)
            for h in range(H):
                if h + 1 < H:
                    emit_qhat(h + 1)
                emit_scores_softmax(h)
                emit_pv(h)

            # batched denominators -> rt = 1/denom  (Ln+Exp, one table set),
            # then an SBUF->SBUF DMA flattens [8, LQ] -> [1, 8*LQ] so each
            # head's row is matmul-addressable at base partition 0.
            lt = fe.tile([1, H * LQ], f32, name="lt", tag="lt", bufs=1)
            nc.scalar.activation(lt[:], dnm[:], AF.Ln, bias=eps30[:])
            rtf = pp.tile([1, H * LQ], bf16, name="rtf", tag="rtf")
            nc.scalar.activation(rtf[:], lt[:], AF.Exp, scale=-1.0)

            # normalize + transpose-accumulate into saT, then x1/sq
            for h in range(H):
                prb = ps_pv.tile([64, LQ], f32, name="prb", tag="pv")
                nc.tensor.matmul(prb[:], onesr_bf[0:1, 0:64],
                                 rtf[0:1, h * LQ:(h + 1) * LQ],
                                 start=True, stop=True)
                rbc = fe.tile([64, LQ], bf16, name="rbc", tag="rbc", bufs=2)
                nc.vector.tensor_copy(rbc[:], prb[:])
                sat = saT[h // 2]
                r0 = (h % 2) * 64
                nc.gpsimd.tensor_tensor(sat[r0:r0 + 64, :], saU[:, h, :],
                                        rbc[:], ALU.mult)
                if h % 2 == 1:
                    dt = h // 2
                    nc.vector.tensor_tensor(x1[dt][:], srcbf[dt][:],
                                            saT[dt][:], ALU.add)
                    nc.gpsimd.tensor_tensor(sq[dt][:], x1[dt][:], x1[dt][:],
                                            ALU.mult)

        # ---- LN1 ----
        def ln_stats_and_apply(xs, sqs, outs, g_sb, b_sb, scope_ps, out_dtype):
            pst = scope_ps.tile([1, 2 * LQ], f32, name="pst", tag="st")
            for dt in range(NDT):
                nc.tensor.matmul(pst[:, 0:LQ], onesc_bf[:], xs[dt][:],
                                 start=(dt == 0), stop=(dt == NDT - 1))
            for dt in range(NDT):
                nc.tensor.matmul(pst[:, LQ:2 * LQ], onesc_bf[:], sqs[dt][:],
                                 start=(dt == 0), stop=(dt == NDT - 1))
            comb = hp.tile([1, 2 * LQ], f32r, name="comb", tag="comb", bufs=1)
            mean = comb[0:1, 0:LQ]
            nc.vector.tensor_scalar(mean, pst[0:1, 0:LQ],
                                    1.0 / D, None, ALU.mult)
            m2 = hp.tile([1, LQ], f32, name="m2", tag="m2")
            nc.vector.tensor_tensor(m2[:], mean.bitcast(f32), mean.bitcast(f32),
                                    ALU.mult)
            var = hp.tile([1, LQ], f32, name="var", tag="var")
            nc.vector.scalar_tensor_tensor(var[:], pst[0:1, LQ:2 * LQ], 1.0 / D,
                                           m2[:], ALU.mult, ALU.subtract)
            lnv = hp.tile([1, LQ], f32, name="lnv", tag="lnv")
            nc.scalar.activation(lnv[:], var[:], AF.Ln, bias=epsln[:])
            nc.scalar.activation(comb[0:1, LQ:2 * LQ], lnv[:],
                                 AF.Exp, scale=-0.5)
            pbc = scope_ps.tile([128, 2 * LQ], f32, name="pbc", tag="bc")
            nc.tensor.matmul(pbc[:, 0:LQ], onesr_fr[:], comb[0:1, 0:LQ],
                             start=True, stop=True)
            nc.tensor.matmul(pbc[:, LQ:2 * LQ], onesr_fr[:], comb[0:1, LQ:2 * LQ],
                             start=True, stop=True)
            mb = hp.tile([128, 2 * LQ], bf16, name="mbrs", tag="mbrs", bufs=1)
            nc.vector.tensor_copy(mb[:], pbc[:])
            for dt in range(NDT):
                t1 = hp.tile([128, LQ], bf16, name="lnt1", tag="lnt1")
                nc.gpsimd.tensor_tensor(t1[:], xs[dt][:], mb[:, 0:LQ],
                                        ALU.subtract)
                if g_sb is None:
                    nc.gpsimd.tensor_tensor(outs[dt][:], t1[:], mb[:, LQ:2 * LQ],
                                            ALU.mult)
                else:
                    t2 = hp.tile([128, LQ], bf16, name="lnt2", tag="lnt2")
                    nc.gpsimd.tensor_tensor(t2[:], t1[:], mb[:, LQ:2 * LQ],
                                            ALU.mult)
                    nc.scalar.activation(outs[dt][:], t2[:], AF.Identity,
                                         bias=b_sb[:, dt:dt + 1],
                                         scale=g_sb[:, dt:dt + 1])

        with ExitStack() as lctx:
            ps_l1 = lctx.enter_context(
                tc.tile_pool(name="psl1", bufs=1, space="PSUM"))
            ln_stats_and_apply(x1, sq, xT, None, None, ps_l1, bf16)

        # ---- FFN ----
        with ExitStack() as fctx:
            ps_f2 = fctx.enter_context(
                tc.tile_pool(name="psf2", bufs=1, space="PSUM"))
            pf2 = [ps_f2.tile([128, LQ], f32, name=f"pf2_{i}", tag=f"f2_{i}")
                   for i in range(NDT)]
            with ExitStack() as hctx:
                ps_h1 = hctx.enter_context(
                    tc.tile_pool(name="psh1", bufs=2, space="PSUM"))
                for p in range(NFT // 2):
                    ph = ps_h1.tile([128, 2 * LQ], f32, name="ph", tag="h1")
                    for half in range(2):
                        ft = 2 * p + half
                        for dt in range(NDT):
                            nc.tensor.matmul(
                                ph[:, half * LQ:(half + 1) * LQ],
                                ff1w[dt][:, ft * 128:(ft + 1) * 128],
                                xT[dt][:],
                                start=(dt == 0), stop=(dt == NDT - 1))
                    h1 = hp.tile([128, 2 * LQ], bf16, name="h1", tag="h1")
                    for half in range(2):
                        ft = 2 * p + half
                        nc.vector.tensor_scalar(
                            h1[:, half * LQ:(half + 1) * LQ],
                            ph[:, half * LQ:(half + 1) * LQ],
                            ff1b[:, ft:ft + 1], 0.0, ALU.add, ALU.max)
                    for half in range(2):
                        ft = 2 * p + half
                        for dot in range(NDT):
                            nc.tensor.matmul(
                                pf2[dot][:],
                                ff2w[ft][:, dot * 128:(dot + 1) * 128],
                                h1[:, half * LQ:(half + 1) * LQ],
                                start=(ft == 0), stop=(ft == NFT - 1))

            # x2 = pf2 + ff2b + x1->xT residual ... residual is xT? No: x2 = x(LN1 out? no)
            with ExitStack() as l2ctx:
                ps_l2 = l2ctx.enter_context(
                    tc.tile_pool(name="psl2", bufs=1, space="PSUM"))
                sq2 = sq  # reuse tiles
                for dot in range(NDT):
                    nc.vector.scalar_tensor_tensor(
                        x2[dot][:], pf2[dot][:], ff2b[:, dot:dot + 1],
                        xT[dot][:], ALU.add, ALU.add)
                    nc.gpsimd.tensor_tensor(sq2[dot][:], x2[dot][:], x2[dot][:],
                                            ALU.mult)
                outs = [hp.tile([128, LQ], f32, name=f"o{dt}", tag=f"o{dt}",
                                bufs=1)
                        for dt in range(NDT)]
                ln_stats_and_apply(x2, sq2, outs, ln2g, ln2b, ps_l2, f32)
                for dt in range(NDT):
                    nc.sync.dma_start(outT_d[dt * 128:(dt + 1) * 128, :],
                                      outs[dt][:])

    nc.compile()
    _PROG_CACHE[key] = nc
    return nc


def _col_tiles(vec):
    """(N,) -> (128, N//128) with [p, j] = vec[j*128 + p]."""
    return np.ascontiguousarray(vec.reshape(-1, 128).T.astype(np.float32))


def _dr_layout(mat):
    """(K, N) -> (128, K//128, N) with [pi, po, n] = mat[po*128+pi, n]."""
    K, N = mat.shape
    return np.ascontiguousarray(
        mat.reshape(K // 128, 128, N).transpose(1, 0, 2))


def _prep_inputs(src, mask, attn_mask, in_proj_w, in_proj_b, ln1_g, ln1_b,
                 ff1_w, ff1_b, ff2_w, ff2_b, ln2_g, ln2_b):
    src = np.asarray(src, np.float32)
    mask = np.asarray(mask, bool)
    attn_mask = np.asarray(attn_mask, bool)
    w = np.asarray(in_proj_w, np.float64)
    b = np.asarray(in_proj_b, np.float64)

    # per-head M = Wq^T Wk (x16 for fp8 range) and u = Wk^T bq (x16)
    m8_rows = []   # [pi, h*NDT+po, d]
    u16_cols = []  # [pi, h*NDT+po]
    for h in range(H):
        wq = w[h * D:(h + 1) * D]                   # (D, D)
        wk = w[(H + h) * D:(H + h + 1) * D]
        bq = b[h * D:(h + 1) * D]
        M = (wq.T @ wk) * SM                        # (D e, D d)
        u = (wk.T @ bq) * SM                        # (D d,)
        m8_rows.append(_dr_layout(M.astype(np.float32)))      # (128, NDT, D)
        u16_cols.append(u.reshape(NDT, 128).T.astype(np.float32))  # (128, NDT)
    m8 = np.concatenate(m8_rows, axis=1).astype(np_f8)  # (128, H*NDT, D)
    u16 = np.concatenate(u16_cols, axis=1)              # (128, H*NDT)

    wv = w[2 * H * D:]                                  # (H*DV, D)
    wv8 = _dr_layout(wv.T.astype(np.float32)).astype(np_f8)  # (128, NDT, H*DV)
    bvbc = np.ascontiguousarray(
        np.broadcast_to(b[2 * H * D:].astype(np.float32), (128, H * DV)))

    shared = {
        "m8": m8, "u16": u16, "wv8": wv8, "bvbc": bvbc,
        "ff1T": np.ascontiguousarray(
            (np.asarray(ff1_w, np.float64) * np.asarray(ln1_g, np.float64)[None, :]
             ).T.astype(np_bf16)),
        "ff2T": np.ascontiguousarray(np.asarray(ff2_w, np.float32).T.astype(np_bf16)),
        "ff1b": _col_tiles(
            (np.asarray(ff1_w, np.float64) @ np.asarray(ln1_b, np.float64)
             + np.asarray(ff1_b, np.float64)).astype(np.float32)),
        "ff2b": _col_tiles(np.asarray(ff2_b, np.float32)),
        "ln2g": _col_tiles(np.asarray(ln2_g, np.float32)),
        "ln2b": _col_tiles(np.asarray(ln2_b, np.float32)),
    }
    in_maps = []
    for c in range(NCORES):
        bidx, half = divmod(c, 2)
        # rotate the token (m) axis so own query tokens are columns 0:LQ
        perm = np.r_[half * LQ:L, 0:half * LQ]
        srcT = src[bidx].T                              # (D, L)
        src8 = _dr_layout(srcT[:, perm]).astype(np_f8)  # (128, NDT, L)
        srcbf = np.ascontiguousarray(
            srcT[:, half * LQ:(half + 1) * LQ].astype(np_bf16))
        # multiplicative mask: [m_rot, l_own] = (1-pad[m]) * (1-attn[l, m])
        cm = ~(mask[bidx][None, :] | attn_mask[half * LQ:(half + 1) * LQ, :])
        cmT = cm[:, perm].T.astype(np.float32)           # (L m_rot, LQ)
        maskf = np.ascontiguousarray(
            cmT.reshape(NMT, 128, LQ).transpose(1, 0, 2)).astype(np_bf16)
        m = dict(shared)
        m["src8"] = src8
        m["srcbf"] = srcbf
        m["maskf"] = maskf
        in_maps.append(m)
    return in_maps


def _run(inputs, trace=False):
    nc = _build_program()
    in_maps = _prep_inputs(**inputs)
    for attempt in range(3):
        try:
            res = run_bass_kernel_spmd(nc, in_maps, list(range(NCORES)),
                                       trace=trace)
            break
        except Exception:  # transient NRT device errors observed
            if attempt == 2:
                raise
    out = np.empty((B, L, D), np.float32)
    for c in range(NCORES):
        bidx, half = divmod(c, 2)
        out[bidx, half * LQ:(half + 1) * LQ, :] = res.results[c]["outT"].T
    return out, res


def kernel(**inputs):
    out, _ = _run(inputs, trace=False)
    return out


if __name__ == "__main__":
    import reference
    inputs = {k: np.asarray(v) for k, v in reference.setup_inputs().items()}
    out = kernel(**inputs)
    print("out", out.shape, out.dtype)


# revision 31
# speedup vs baseline: 1.9524x; 1.1224x over previous
"""Trainium2 Bass kernel for an AttNHP transformer layer (v2, fp8 DoubleRow).

Shapes (hardcoded): src (4, 1024, 512), nhead=8 with full-width (512) q/k per
head, dim_value 64, ffn 2048.  SPMD on 8 NeuronCores: core c handles batch
c//2, query-token half c%2.

Key algebra (host-folded):
  scores[l,m] = q_l.k_m = s_l^T (Wq^T Wk) s_m + (Wk^T bq).s_m + f(l)
The f(l) terms shift all logits of a query equally and cancel in softmax, so
with M_h = Wq_h^T Wk_h and u_h = Wk_h^T bq_h (both host-precomputed):
  qhat_l = M_h^T-applied projection of own queries (+u_h bias), keys enter as
  raw src.  This removes the Q/K projections entirely.  qhat/scores/V run as
fp8e4 DoubleRow matmuls (2x PE rate, 256-deep contraction per pass); the
pad+attn mask is one multiplicative bf16 {0,1} tensor applied post-exp; FFN
runs in bf16.  Per-head softmax denominators are batched into one Ln/Exp pair.
"""

import os
import sys
import numpy as np
from contextlib import ExitStack

for _p in ("/opt/trn_rl_repo", "/root/.axon_site/_ro/trn_rl_repo"):
    if os.path.isdir(_p) and _p not in sys.path:
        sys.path.append(_p)

import ml_dtypes  # noqa: E402
import concourse.bass as bass  # noqa: E402
import concourse.tile as tile  # noqa: E402
from concourse import bacc, mybir  # noqa: E402
from concourse.bass_utils import run_bass_kernel_spmd  # noqa: E402

f32 = mybir.dt.float32
f32r = mybir.dt.float32r
bf16 = mybir.dt.bfloat16
f8 = mybir.dt.float8e4
AF = mybir.ActivationFunctionType
ALU = mybir.AluOpType
DR = mybir.MatmulPerfMode.DoubleRow
np_f8 = ml_dtypes.float8_e4m3fn
np_bf16 = ml_dtypes.bfloat16

B, L, D, H, DV, F = 4, 1024, 512, 8, 64, 2048
LQ = L // 2          # own query tokens per core
NCORES = 8
NDT = D // 128       # 4
NMT = L // 128       # 8
NFT = F // 128       # 16
SM = 16.0            # host prescale on M (fp8 range)
SCALE = 1.0 / (SM * float(np.sqrt(np.float32(D))))
NINF = -1000000.0

_PROG_CACHE = {}
_ACT_PATCHED = [False]


def _patch_act_tables():
    """Force every ACTIVATE onto natural_log_exp_and_others (one table load)."""
    if _ACT_PATCHED[0]:
        return
    orig = bacc.get_activation_tables

    def patched(arch):
        t = dict(orig(arch))
        keep = t.get("natural_log_exp_and_others")
        if not keep:
            return t
        for k in t:
            if k != "natural_log_exp_and_others":
                t[k] = t[k] - keep
        return t

    bacc.get_activation_tables = patched
    _ACT_PATCHED[0] = True


def _build_program():
    key = "prog"
    if key in _PROG_CACHE:
        return _PROG_CACHE[key]
    _patch_act_tables()

    nc = bacc.Bacc("TRN2", target_bir_lowering=False, debug=False,
                   num_devices=NCORES)

    def din(name, shape, dt):
        return nc.dram_tensor(name, list(shape), dt, kind="ExternalInput").ap()

    src8_d = din("src8", (128, NDT, L), f8)         # [pi, po, m] = src[po*128+pi, m]
    srcbf_d = din("srcbf", (D, LQ), bf16)           # own-half srcT for residual
    m8_d = din("m8", (128, H * NDT, D), f8)         # per head: [pi, po, d] of 16*M_h
    u16_d = din("u16", (128, H * NDT), f32)         # 16*u_h[po*128+pi]
    maskf_d = din("maskf", (128, NMT, LQ), bf16)    # (1-pad)(1-attn) as {0,1}
    wv8_d = din("wv8", (128, NDT, H * DV), f8)      # [pi, po, j]
    bvbc_d = din("bvbc", (128, H * DV), f32)
    ff1T_d = din("ff1T", (D, F), bf16)
    ff2T_d = din("ff2T", (F, D), bf16)
    ff1b_d = din("ff1b", (128, NFT), f32)
    ff2b_d = din("ff2b", (128, NDT), f32)
    ln2g_d = din("ln2g", (128, NDT), f32)
    ln2b_d = din("ln2b", (128, NDT), f32)
    outT_d = nc.dram_tensor("outT", [D, LQ], f32, kind="ExternalOutput").ap()

    with tile.TileContext(nc) as tc, ExitStack() as ctx:
        pp = ctx.enter_context(tc.tile_pool(name="pp", bufs=1))
        hp = ctx.enter_context(tc.tile_pool(name="hp", bufs=2))

        def load_const(name, dram, shape, dt=f32):
            t = pp.tile(list(shape), dt, name=name, tag=name)
            nc.sync.dma_start(t[:], dram[:])
            return t

        # early loads: src8 + wv8 + first M head gate the PE start
        src8 = pp.tile([128, NDT, L], f8, name="src8", tag="src8")
        nc.sync.dma_start(src8[:], src8_d[:])
        wv8 = pp.tile([128, NDT, H * DV], f8, name="wv8", tag="wv8")
        nc.sync.dma_start(wv8[:], wv8_d[:])
        m8 = pp.tile([128, H * NDT, D], f8, name="m8", tag="m8")
        nc.sync.dma_start(m8[:, 0:NDT, :], m8_d[:, 0:NDT, :])
        u16 = load_const("u16", u16_d, (128, H * NDT))
        bvbc = load_const("bvbc", bvbc_d, (128, H * DV))

        # small consts
        onesc_bf = pp.tile([128, 1], bf16, name="onescb", tag="onescb")
        nc.vector.memset(onesc_bf[:], 1.0)
        onesr_f = pp.tile([1, 128], f32, name="onesrw", tag="onesrw")
        nc.vector.memset(onesr_f[:], 1.0)
        onesr_fr = pp.tile([1, 128], f32r, name="onesrf", tag="onesrf")
        nc.vector.tensor_copy(onesr_fr[:], onesr_f[:])
        onesr_bf = pp.tile([1, 128], bf16, name="onesrb", tag="onesrb")
        nc.vector.memset(onesr_bf[:], 1.0)
        eps30 = pp.tile([1, 1], f32, name="eps30", tag="eps30")
        nc.vector.memset(eps30[:], 1e-30)
        epsln = pp.tile([1, 1], f32, name="epsln", tag="epsln")
        nc.vector.memset(epsln[:], 1e-5)

        # second M head, then the bulkier mask while head 0 computes
        nc.sync.dma_start(m8[:, NDT:2 * NDT, :], m8_d[:, NDT:2 * NDT, :])
        maskf = pp.tile([128, NMT, LQ], bf16, name="maskf", tag="maskf")
        nc.sync.dma_start(maskf[:], maskf_d[:])
        srcbf = [pp.tile([128, LQ], bf16, name=f"srcbf{dt}", tag=f"srcbf{dt}")
                 for dt in range(NDT)]
        for dt in range(NDT):
            nc.sync.dma_start(srcbf[dt][:],
                              srcbf_d[dt * 128:(dt + 1) * 128, :])
        # remaining M heads
        nc.sync.dma_start(m8[:, 2 * NDT:H * NDT, :], m8_d[:, 2 * NDT:H * NDT, :])

        # vaug: [pi, mt, h*65+c]; c==64 is the ones column (softmax denom)
        vaug = pp.tile([128, NMT, H * 65], bf16, name="vaug", tag="vaug")
        v_ones = vaug[:].rearrange("p m (h c) -> p m h c", c=65)[:, :, :, 64:65]
        nc.vector.memset(v_ones, 1.0)

        # persistent activations
        qhat8 = [pp.tile([128, NDT, LQ], f8, name=f"qh8_{i}", tag=f"qh8_{i}")
                 for i in range(2)]
        ex = [pp.tile([128, NMT, LQ], bf16, name=f"ex{i}", tag=f"ex{i}")
              for i in range(2)]
        saU = pp.tile([64, H, LQ], bf16, name="saU", tag="saU")
        dnm = pp.tile([1, H * LQ], bf16, name="dnm", tag="dnm")
        saT = [pp.tile([128, LQ], bf16, name=f"saT{dt}", tag=f"saT{dt}")
               for dt in range(NDT)]
        x1 = [pp.tile([128, LQ], bf16, name=f"x1_{dt}", tag=f"x1_{dt}")
              for dt in range(NDT)]
        sq = [pp.tile([128, LQ], bf16, name=f"sq{dt}", tag=f"sq{dt}")
              for dt in range(NDT)]
        xT = [pp.tile([128, LQ], bf16, name=f"xT{dt}", tag=f"xT{dt}")
              for dt in range(NDT)]
        x2 = [pp.tile([128, LQ], bf16, name=f"x2_{dt}", tag=f"x2_{dt}")
              for dt in range(NDT)]

        # FFN weights (prefetched; consumed after attention)
        ff1w = [pp.tile([128, F], bf16, name=f"f1w{dt}", tag=f"f1w{dt}")
                for dt in range(NDT)]
        ff2w = [pp.tile([128, D], bf16, name=f"f2w{ft}", tag=f"f2w{ft}")
                for ft in range(NFT)]
        for dt in range(NDT):
            nc.sync.dma_start(ff1w[dt][:], ff1T_d[dt * 128:(dt + 1) * 128, :])
        for ft in range(NFT):
            nc.sync.dma_start(ff2w[ft][:], ff2T_d[ft * 128:(ft + 1) * 128, :])
        ff1b = load_const("ff1b", ff1b_d, (128, NFT))
        ff2b = load_const("ff2b", ff2b_d, (128, NDT))
        ln2g = load_const("ln2g", ln2g_d, (128, NDT))
        ln2b = load_const("ln2b", ln2b_d, (128, NDT))

        with ExitStack() as actx:
            ps_qh = actx.enter_context(
                tc.tile_pool(name="psqh", bufs=2, space="PSUM"))
            ps_sc = actx.enter_context(
                tc.tile_pool(name="pssc", bufs=2, space="PSUM"))
            ps_pv = actx.enter_context(
                tc.tile_pool(name="pspv", bufs=2, space="PSUM"))
            fe = actx.enter_context(tc.tile_pool(name="fe", bufs=3))

            def emit_qhat(h):
                """qhat8[h%2][:, po, :] = fp8(psum + 16*u), psum = 16*M_h^T s_own.

                Host rotates the m axis so own query tokens are columns 0:LQ.
                """
                q8 = qhat8[h % 2]
                for po in range(NDT):
                    pq = ps_qh.tile([128, LQ], f32, name="pq", tag="qh")
                    for t in range(NDT // 2):
                        nc.tensor.matmul(
                            pq[:],
                            m8[:, h * NDT + 2 * t:h * NDT + 2 * t + 2,
                               po * 128:(po + 1) * 128],
                            src8[:, 2 * t:2 * t + 2, 0:LQ],
                            start=(t == 0), stop=(t == NDT // 2 - 1),
                            perf_mode=DR)
                    nc.vector.tensor_scalar(
                        q8[:, po, :], pq[:],
                        u16[:, h * NDT + po:h * NDT + po + 1], None, ALU.add)

            def emit_scores_softmax(h):
                """psc pairs -> exp (ACT) -> mask mult (DVE) -> ex[h%2]."""
                exh = ex[h % 2]
                q8 = qhat8[h % 2]
                for p in range(NMT // 2):
                    psc = ps_sc.tile([128, 2 * LQ], f32, name="psc", tag="sc")
                    for half in range(2):
                        mt = 2 * p + half
                        for t in range(NDT // 2):
                            nc.tensor.matmul(
                                psc[:, half * LQ:(half + 1) * LQ],
                                src8[:, 2 * t:2 * t + 2,
                                     mt * 128:(mt + 1) * 128],
                                q8[:, 2 * t:2 * t + 2, :],
                                start=(t == 0), stop=(t == NDT // 2 - 1),
                                perf_mode=DR)
                    exf = fe.tile([128, 2 * LQ], bf16, name="exf", tag="exf")
                    nc.scalar.activation(exf[:], psc[:], AF.Exp, scale=SCALE)
                    nc.vector.tensor_tensor(
                        exh[:, 2 * p:2 * p + 2, :].rearrange("p a b -> p (a b)"),
                        exf[:], maskf[:, 2 * p:2 * p + 2, :].rearrange(
                            "p a b -> p (a b)"), ALU.mult)

            def emit_pv(h):
                """ppv[65, LQ]: rows 0:64 numerator, row 64 denominator."""
                exh = ex[h % 2]
                ppv = ps_pv.tile([65, LQ], f32, name="ppv", tag="pv")
                for mt in range(NMT):
                    nc.tensor.matmul(
                        ppv[:],
                        vaug[:, mt, h * 65:(h + 1) * 65],
                        exh[:, mt, :],
                        start=(mt == 0), stop=(mt == NMT - 1))
                nc.vector.tensor_copy(saU[:, h, :], ppv[0:64, :])
                nc.vector.tensor_copy(dnm[0:1, h * LQ:(h + 1) * LQ],
                                      ppv[64:65, :])

            # V projection (into vaug) -- uses the sc psum pairs before heads
            for p in range(NMT // 2):
                pv = ps_sc.tile([128, 2 * LQ], f32, name="pvv", tag="sc")
                for half in range(2):
                    mt = 2 * p + half
                    for t in range(NDT // 2):
                        nc.tensor.matmul(
                            pv[:, half * LQ:(half + 1) * LQ],
                            src8[:, 2 * t:2 * t + 2, mt * 128:(mt + 1) * 128],
                            wv8[:, 2 * t:2 * t + 2, :],
                            start=(t == 0), stop=(t == NDT // 2 - 1),
                            perf_mode=DR)
                for half in range(2):
                    mt = 2 * p + half
                    va_v = vaug[:, mt, :].rearrange(
                        "p (h c) -> p h c", c=65)[:, :, 0:64]
                    pv_v = pv[:, half * LQ:(half + 1) * LQ].rearrange(
                        "p (h c) -> p h c", c=64)
                    bv_v = bvbc[:].rearrange("p (h c) -> p h c", c=64)
                    nc.vector.tensor_tensor(va_v, pv_v, bv_v, ALU.add)

            rtf = pp.tile([1, H * LQ], bf16, name="rtf", tag="rtf")
            lt = fe.tile([1, H * LQ], f32, name="lt", tag="lt", bufs=1)

            def emit_norm_chunk(dt):
                """Normalize heads (2dt, 2dt+1) -> saT[dt] -> x1/sq[dt].

                Runs on ACT/DVE/GPSIMD while PE continues with later heads.
                """
                s = 2 * dt * LQ
                nc.scalar.activation(lt[0:1, s:s + 2 * LQ],
                                     dnm[0:1, s:s + 2 * LQ], AF.Ln,
                                     bias=eps30[:])
                nc.scalar.activation(rtf[0:1, s:s + 2 * LQ],
                                     lt[0:1, s:s + 2 * LQ], AF.Exp, scale=-1.0)
                for h in (2 * dt, 2 * dt + 1):
                    prb = ps_pv.tile([64, LQ], f32, name="prb", tag="pv")
                    nc.tensor.matmul(prb[:], onesr_bf[0:1, 0:64],
                                     rtf[0:1, h * LQ:(h + 1) * LQ],
                                     start=True, stop=True)
                    rbc = fe.tile([64, LQ], bf16, name="rbc", tag="rbc", bufs=2)
                    nc.vector.tensor_copy(rbc[:], prb[:])
                    r0 = (h % 2) * 64
                    nc.gpsimd.tensor_tensor(saT[dt][r0:r0 + 64, :],
                                            saU[:, h, :], rbc[:], ALU.mult)
                nc.vector.tensor_tensor(x1[dt][:], srcbf[dt][:], saT[dt][:],
                                        ALU.add)
                nc.gpsimd.tensor_tensor(sq[dt][:], x1[dt][:], x1[dt][:],
                                        ALU.mult)

            # head pipeline: qhat(h+1) first so its DVE quantize lands ahead
            # of mask(h) in the DVE queue (in-order engines); norm chunk for
            # head pair (h-1, h) overlaps the next heads' attention.
            emit_qhat(0)
            for h in range(H):
                if h + 1 < H:
                    emit_qhat(h + 1)
                emit_scores_softmax(h)
                emit_pv(h)
                if h % 2 == 1:
                    emit_norm_chunk(h // 2)

        # ---- LN1 ----
        def ln_stats_and_apply(xs, sqs, outs, g_sb, b_sb, scope_ps, out_dtype):
            pst = scope_ps.tile([1, 2 * LQ], f32, name="pst", tag="st")
            for dt in range(NDT):
                nc.tensor.matmul(pst[:, 0:LQ], onesc_bf[:], xs[dt][:],
                                 start=(dt == 0), stop=(dt == NDT - 1))
            for dt in range(NDT):
                nc.tensor.matmul(pst[:, LQ:2 * LQ], onesc_bf[:], sqs[dt][:],
                                 start=(dt == 0), stop=(dt == NDT - 1))
            comb = hp.tile([1, 2 * LQ], f32r, name="comb", tag="comb", bufs=1)
            mean = comb[0:1, 0:LQ]
            nc.vector.tensor_scalar(mean, pst[0:1, 0:LQ],
                                    1.0 / D, None, ALU.mult)
            m2 = hp.tile([1, LQ], f32, name="m2", tag="m2")
            nc.vector.tensor_tensor(m2[:], mean.bitcast(f32), mean.bitcast(f32),
                                    ALU.mult)
            var = hp.tile([1, LQ], f32, name="var", tag="var")
            nc.vector.scalar_tensor_tensor(var[:], pst[0:1, LQ:2 * LQ], 1.0 / D,
                                           m2[:], ALU.mult, ALU.subtract)
            lnv = hp.tile([1, LQ], f32, name="lnv", tag="lnv")
            nc.scalar.activation(lnv[:], var[:], AF.Ln, bias=epsln[:])
            nc.scalar.activation(comb[0:1, LQ:2 * LQ], lnv[:],
                                 AF.Exp, scale=-0.5)
            pbc = scope_ps.tile([128, 2 * LQ], f32, name="pbc", tag="bc")
            nc.tensor.matmul(pbc[:, 0:LQ], onesr_fr[:], comb[0:1, 0:LQ],
                             start=True, stop=True)
            nc.tensor.matmul(pbc[:, LQ:2 * LQ], onesr_fr[:], comb[0:1, LQ:2 * LQ],
                             start=True, stop=True)
            mb = hp.tile([128, 2 * LQ], bf16, name="mbrs", tag="mbrs", bufs=1)
            nc.vector.tensor_copy(mb[:], pbc[:])
            for dt in range(NDT):
                # alternate engines so the four applies run two-wide
                eng = nc.vector if dt % 2 == 0 else nc.gpsimd
                t1 = hp.tile([128, LQ], bf16, name="lnt1", tag="lnt1")
                eng.tensor_tensor(t1[:], xs[dt][:], mb[:, 0:LQ], ALU.subtract)
                if g_sb is None:
                    eng.tensor_tensor(outs[dt][:], t1[:], mb[:, LQ:2 * LQ],
                                      ALU.mult)
                else:
                    t2 = hp.tile([128, LQ], bf16, name="lnt2", tag="lnt2")
                    eng.tensor_tensor(t2[:], t1[:], mb[:, LQ:2 * LQ], ALU.mult)
                    nc.scalar.activation(outs[dt][:], t2[:], AF.Identity,
                                         bias=b_sb[:, dt:dt + 1],
                                         scale=g_sb[:, dt:dt + 1])

        with ExitStack() as lctx:
            ps_l1 = lctx.enter_context(
                tc.tile_pool(name="psl1", bufs=1, space="PSUM"))
            ln_stats_and_apply(x1, sq, xT, None, None, ps_l1, bf16)

        # ---- FFN ----
        with ExitStack() as fctx:
            ps_f2 = fctx.enter_context(
                tc.tile_pool(name="psf2", bufs=1, space="PSUM"))
            pf2 = [ps_f2.tile([128, LQ], f32, name=f"pf2_{i}", tag=f"f2_{i}")
                   for i in range(NDT)]
            with ExitStack() as hctx:
                ps_h1 = hctx.enter_context(
                    tc.tile_pool(name="psh1", bufs=2, space="PSUM"))
                for p in range(NFT // 2):
                    ph = ps_h1.tile([128, 2 * LQ], f32, name="ph", tag="h1")
                    for half in range(2):
                        ft = 2 * p + half
                        for dt in range(NDT):
                            nc.tensor.matmul(
                                ph[:, half * LQ:(half + 1) * LQ],
                                ff1w[dt][:, ft * 128:(ft + 1) * 128],
                                xT[dt][:],
                                start=(dt == 0), stop=(dt == NDT - 1))
                    h1 = hp.tile([128, 2 * LQ], bf16, name="h1", tag="h1")
                    for half in range(2):
                        ft = 2 * p + half
                        nc.vector.tensor_scalar(
                            h1[:, half * LQ:(half + 1) * LQ],
                            ph[:, half * LQ:(half + 1) * LQ],
                            ff1b[:, ft:ft + 1], 0.0, ALU.add, ALU.max)
                    for half in range(2):
                        ft = 2 * p + half
                        for dot in range(NDT):
                            nc.tensor.matmul(
                                pf2[dot][:],
                                ff2w[ft][:, dot * 128:(dot + 1) * 128],
                                h1[:, half * LQ:(half + 1) * LQ],
                                start=(ft == 0), stop=(ft == NFT - 1))

            # x2 = pf2 + ff2b + x1->xT residual ... residual is xT? No: x2 = x(LN1 out? no)
            with ExitStack() as l2ctx:
                ps_l2 = l2ctx.enter_context(
                    tc.tile_pool(name="psl2", bufs=1, space="PSUM"))
                sq2 = sq  # reuse tiles
                for dot in range(NDT):
                    nc.vector.scalar_tensor_tensor(
                        x2[dot][:], pf2[dot][:], ff2b[:, dot:dot + 1],
                        xT[dot][:], ALU.add, ALU.add)
                    eng = nc.gpsimd if dot % 2 == 0 else nc.vector
                    eng.tensor_tensor(sq2[dot][:], x2[dot][:], x2[dot][:],
                                      ALU.mult)
                outs = [hp.tile([128, LQ], f32, name=f"o{dt}", tag=f"o{dt}",
                                bufs=1)
                        for dt in range(NDT)]
                ln_stats_and_apply(x2, sq2, outs, ln2g, ln2b, ps_l2, f32)
                for dt in range(NDT):
                    nc.sync.dma_start(outT_d[dt * 128:(dt + 1) * 128, :],
                                      outs[dt][:])

    nc.compile()
    _PROG_CACHE[key] = nc
    return nc


def _col_tiles(vec):
    """(N,) -> (128, N//128) with [p, j] = vec[j*128 + p]."""
    return np.ascontiguousarray(vec.reshape(-1, 128).T.astype(np.float32))


def _dr_layout(mat):
    """(K, N) -> (128, K//128, N) with [pi, po, n] = mat[po*128+pi, n]."""
    K, N = mat.shape
    return np.ascontiguousarray(
        mat.reshape(K // 128, 128, N).transpose(1, 0, 2))


def _prep_inputs(src, mask, attn_mask, in_proj_w, in_proj_b, ln1_g, ln1_b,
                 ff1_w, ff1_b, ff2_w, ff2_b, ln2_g, ln2_b):
    src = np.asarray(src, np.float32)
    mask = np.asarray(mask, bool)
    attn_mask = np.asarray(attn_mask, bool)
    w = np.asarray(in_proj_w, np.float64)
    b = np.asarray(in_proj_b, np.float64)

    # per-head M = Wq^T Wk (x16 for fp8 range) and u = Wk^T bq (x16)
    m8_rows = []   # [pi, h*NDT+po, d]
    u16_cols = []  # [pi, h*NDT+po]
    for h in range(H):
        wq = w[h * D:(h + 1) * D]                   # (D, D)
        wk = w[(H + h) * D:(H + h + 1) * D]
        bq = b[h * D:(h + 1) * D]
        M = (wq.T @ wk) * SM                        # (D e, D d)
        u = (wk.T @ bq) * SM                        # (D d,)
        m8_rows.append(_dr_layout(M.astype(np.float32)))      # (128, NDT, D)
        u16_cols.append(u.reshape(NDT, 128).T.astype(np.float32))  # (128, NDT)
    m8 = np.concatenate(m8_rows, axis=1).astype(np_f8)  # (128, H*NDT, D)
    u16 = np.concatenate(u16_cols, axis=1)              # (128, H*NDT)

    wv = w[2 * H * D:]                                  # (H*DV, D)
    wv8 = _dr_layout(wv.T.astype(np.float32)).astype(np_f8)  # (128, NDT, H*DV)
    bvbc = np.ascontiguousarray(
        np.broadcast_to(b[2 * H * D:].astype(np.float32), (128, H * DV)))

    shared = {
        "m8": m8, "u16": u16, "wv8": wv8, "bvbc": bvbc,
        "ff1T": np.ascontiguousarray(
            (np.asarray(ff1_w, np.float64) * np.asarray(ln1_g, np.float64)[None, :]
             ).T.astype(np_bf16)),
        "ff2T": np.ascontiguousarray(np.asarray(ff2_w, np.float32).T.astype(np_bf16)),
        "ff1b": _col_tiles(
            (np.asarray(ff1_w, np.float64) @ np.asarray(ln1_b, np.float64)
             + np.asarray(ff1_b, np.float64)).astype(np.float32)),
        "ff2b": _col_tiles(np.asarray(ff2_b, np.float32)),
        "ln2g": _col_tiles(np.asarray(ln2_g, np.float32)),
        "ln2b": _col_tiles(np.asarray(ln2_b, np.float32)),
    }
    in_maps = []
    for c in range(NCORES):
        bidx, half = divmod(c, 2)
        # rotate the token (m) axis so own query tokens are columns 0:LQ
        perm = np.r_[half * LQ:L, 0:half * LQ]
        srcT = src[bidx].T                              # (D, L)
        src8 = _dr_layout(srcT[:, perm]).astype(np_f8)  # (128, NDT, L)
        srcbf = np.ascontiguousarray(
            srcT[:, half * LQ:(half + 1) * LQ].astype(np_bf16))
        # multiplicative mask: [m_rot, l_own] = (1-pad[m]) * (1-attn[l, m])
        cm = ~(mask[bidx][None, :] | attn_mask[half * LQ:(half + 1) * LQ, :])
        cmT = cm[:, perm].T.astype(np.float32)           # (L m_rot, LQ)
        maskf = np.ascontiguousarray(
            cmT.reshape(NMT, 128, LQ).transpose(1, 0, 2)).astype(np_bf16)
        m = dict(shared)
        m["src8"] = src8
        m["srcbf"] = srcbf
        m["maskf"] = maskf
        in_maps.append(m)
    return in_maps


def _run(inputs, trace=False):
    nc = _build_program()
    in_maps = _prep_inputs(**inputs)
    for attempt in range(3):
        try:
            res = run_bass_kernel_spmd(nc, in_maps, list(range(NCORES)),
                                       trace=trace)
            break
        except Exception:  # transient NRT device errors observed
            if attempt == 2:
                raise
    out = np.empty((B, L, D), np.float32)
    for c in range(NCORES):
        bidx, half = divmod(c, 2)
        out[bidx, half * LQ:(half + 1) * LQ, :] = res.results[c]["outT"].T
    return out, res


def kernel(**inputs):
    out, _ = _run(inputs, trace=False)
    return out


if __name__ == "__main__":
    import reference
    inputs = {k: np.asarray(v) for k, v in reference.setup_inputs().items()}
    out = kernel(**inputs)
    print("out", out.shape, out.dtype)
